# revision 1
# baseline (speedup 1.0000x reference)
"""DAHH hypergraph conv (gnn_message_passing) on 8 Trainium2 NeuronCores.

reference:
    xp      = x @ theta                      [N, 64]
    de      = colsum(H)                      [E]
    edge_ft = (H.T @ xp) / de[:, None]       [E, 64]
    dn      = rowsum(H)                      [N]
    node_ft = (H @ edge_ft) / dn[:, None]    [N, 64]

Sharding: H and x row-sharded (node dim) across 8 cores; theta replicated.
Per core:
  phase 0: xp1[k] = [x_shard @ theta | 1]  (PE transpose of x tiles + matmul)
  phase 1: partial edge sums  P = H_shard.T @ xp1  as [65, e] psum chunks
           (lhsT=xp1 bf16, rhs=H cast to bf16 on DVE/ACT, full PE rate),
           transposed on PE into [e, 65] tiles, staged to DRAM bounce.
  phase 2: AllReduce (sum partial edge sums over the 8 node shards).
  phase 3: edge_ft1[e] = [P[e,0:64]/max(P[e,64],eps) | 1]  (per-partition
           scalar ops; edges on partitions).
  phase 4: node out: po = sum_e H.T_tile[e,n].T @ eft1[e]  -> [n, 65];
           H tiles PE-transposed on the fly (f32), cast to bf16 for the
           matmul (FWL weight loads); out = po[:,0:64]/max(po[:,64],eps).

All H traffic is HWDGE f32 (measured ~3x faster than SWDGE cast DMA).
N padded 20000->20480 (2560/core), E padded 10000->10240 with zeros; padding
is numerically inert (zero rows/cols, degree clamps via max(.,1e-20)).
"""
import sys
sys.path.insert(0, "/opt/trn_rl_repo")
import numpy as np

import concourse.bass as bass
import concourse.bacc as bacc
import concourse.tile as tile
import concourse.mybir as mybir
from concourse.bass_utils import run_bass_kernel_spmd

N, E, IN_CH, OUT_CH = 20000, 10000, 128, 64
N_CORES = 8
NL = 2560            # padded nodes per core
EP = 10240           # padded edges
NT = NL // 128       # 20 node tiles per core
ET = EP // 128       # 80 edge tiles
CC = 2048            # H DMA chunk width (e cols)
NCC = EP // CC       # 5
W = 65               # 64 features + degree/ones column

f32 = mybir.dt.float32
f32r = mybir.dt.float32r
bf16 = mybir.dt.bfloat16

SKIP_COLLECTIVE = False   # dev-only: replace AllReduce with local copy


def build_body(nc, tc, x_ext, H_ext, th_ext, id_ext, out_ext, sfx="",
               phases=(0, 1, 2, 3, 4), dma_only=False):
    with (
        tc.tile_pool(name=f"const{sfx}", bufs=1) as constp,
        tc.tile_pool(name=f"persist{sfx}", bufs=1) as persist,
        tc.tile_pool(name=f"dram{sfx}", bufs=1, space="DRAM") as dram,
    ):
        ident = constp.tile([128, 128], f32)
        nc.sync.dma_start(ident[:], id_ext[:])
        th_f = constp.tile([128, OUT_CH], f32)
        nc.sync.dma_start(th_f[:], th_ext[:])
        th_b = constp.tile([128, OUT_CH], bf16)
        nc.vector.tensor_copy(th_b[:], th_f[:])
        acc = constp.tile([128, 1], f32)   # dma_only sink
        nc.vector.memset(acc[:], 0.0)

        xp1 = persist.tile([128, NT * W], bf16)
        eft1 = persist.tile([128, ET * W], bf16)
        bounce_in = dram.tile([128, ET * W], f32)
        bounce_out = dram.tile([128, ET * W], f32, addr_space="Shared")

        # ---- phase 0: xp1 = [x @ theta | 1] per node tile ----
        if 0 in phases:
            with (
                tc.tile_pool(name=f"p0{sfx}", bufs=3) as p0,
                tc.tile_pool(name=f"p0ps{sfx}", bufs=2, space="PSUM") as p0ps,
            ):
                for k in range(NT):
                    xt = p0.tile([128, 128], f32, tag="x")
                    nc.sync.dma_start(xt[:], x_ext[k * 128:(k + 1) * 128, :])
                    pt = p0ps.tile([128, 128], f32, tag="pt")
                    nc.tensor.transpose(pt[:], xt[:], ident[:])
                    xT = p0.tile([128, 128], bf16, tag="xT")
                    nc.vector.tensor_copy(xT[:], pt[:])
                    pxp = p0ps.tile([128, OUT_CH], f32, tag="pxp")
                    nc.tensor.matmul(pxp[:], xT[:], th_b[:], start=True, stop=True)
                    nc.vector.tensor_copy(xp1[:, k * W:k * W + OUT_CH], pxp[:])
                    nc.vector.memset(xp1[:, k * W + OUT_CH:(k + 1) * W], 1.0)

        # ---- phase 1: partial edge sums, transposed to [e, 65] tiles ----
        if 1 in phases:
            with (
                tc.tile_pool(name=f"p1stage{sfx}", bufs=1) as p1stage,
                tc.tile_pool(name=f"p1h{sfx}", bufs=5) as p1h,
                tc.tile_pool(name=f"p1ps{sfx}", bufs=1, space="PSUM") as p1ps,
                tc.tile_pool(name=f"p1e{sfx}", bufs=3) as p1e,
                tc.tile_pool(name=f"p1ps2{sfx}", bufs=2, space="PSUM") as p1ps2,
            ):
                ar_in = p1stage.tile([128, ET * W], f32)
                for cc in range(NCC):
                    psums = [p1ps.tile([W, 512], f32, tag=f"pch{j}",
                                       name=f"pch{j}_{cc}{sfx}")
                             for j in range(4)]
                    for k in range(NT):
                        h = p1h.tile([128, CC], f32, tag="h")
                        nc.sync.dma_start(
                            h[:], H_ext[k * 128:(k + 1) * 128, cc * CC:(cc + 1) * CC])
                        if dma_only:
                            nc.vector.tensor_tensor(
                                acc[:], acc[:], h[:, 0:1], mybir.AluOpType.add)
                            continue
                        hb = p1h.tile([128, CC], bf16, tag="hb")
                        if k % 2 == 0:
                            nc.vector.tensor_copy(hb[:], h[:])
                        else:
                            nc.scalar.activation(
                                hb[:], h[:], mybir.ActivationFunctionType.Copy)
                        for j in range(4):
                            nc.tensor.matmul(
                                psums[j][:],
                                xp1[:, k * W:(k + 1) * W],
                                hb[:, j * 512:(j + 1) * 512],
                                start=(k == 0), stop=(k == NT - 1))
                    if dma_only:
                        continue
                    for j in range(4):
                        et = p1e.tile([W, 512], f32, tag="et")
                        nc.vector.tensor_copy(et[:], psums[j][:])
                        for q in range(4):
                            t_idx = cc * 16 + j * 4 + q
                            ptr = p1ps2.tile([128, W], f32, tag="ptr")
                            nc.tensor.transpose(
                                ptr[:], et[:, q * 128:(q + 1) * 128], ident[0:W, 0:W])
                            nc.vector.tensor_copy(
                                ar_in[:, t_idx * W:(t_idx + 1) * W], ptr[:])
                if not dma_only:
                    nc.sync.dma_start(bounce_in[:], ar_in[:])

        # ---- phase 2: AllReduce over the 8 node shards ----
        if 2 in phases:
            if SKIP_COLLECTIVE:
                nc.sync.dma_start(bounce_out[:], bounce_in[:])
            else:
                nc.gpsimd.collective_compute(
                    "AllReduce", mybir.AluOpType.add,
                    replica_groups=[list(range(N_CORES))],
                    ins=[bounce_in.opt()], outs=[bounce_out.opt()])

        # ---- phase 3: normalize edge features; append ones column ----
        if 3 in phases:
            with (
                tc.tile_pool(name=f"p3stage{sfx}", bufs=1) as p3stage,
                tc.tile_pool(name=f"p3{sfx}", bufs=2) as p3,
            ):
                eftf = p3stage.tile([128, ET * W], f32)
                if 2 in phases:
                    nc.sync.dma_start(eftf[:], bounce_out[:])
                else:
                    nc.vector.memset(eftf[:], 1.0)
                for t in range(ET):
                    b = t * W
                    de = p3.tile([128, 1], f32, tag="de")
                    nc.vector.tensor_scalar_max(de[:], eftf[:, b + OUT_CH:b + W], 1e-20)
                    rec = p3.tile([128, 1], f32, tag="rec")
                    nc.vector.reciprocal(rec[:], de[:])
                    nc.vector.tensor_scalar_mul(
                        eft1[:, b:b + OUT_CH], eftf[:, b:b + OUT_CH], rec[:])
                    nc.vector.memset(eft1[:, b + OUT_CH:b + W], 1.0)

        # ---- phase 4: node aggregation with on-the-fly H transpose ----
        if 4 in phases:
            with (
                tc.tile_pool(name=f"p4h{sfx}", bufs=4) as p4h,
                tc.tile_pool(name=f"p4t{sfx}", bufs=32) as p4t,
                tc.tile_pool(name=f"p4ps{sfx}", bufs=3, space="PSUM") as p4ps,
                tc.tile_pool(name=f"p4po{sfx}", bufs=2, space="PSUM") as p4po,
                tc.tile_pool(name=f"p4o{sfx}", bufs=3) as p4o,
            ):
                for nt in range(NT):
                    po = p4po.tile([128, W], f32, tag="po")
                    for cc in range(NCC):
                        h = p4h.tile([128, CC], f32, tag="h")
                        nc.sync.dma_start(
                            h[:], H_ext[nt * 128:(nt + 1) * 128, cc * CC:(cc + 1) * CC])
                        if dma_only:
                            nc.vector.tensor_tensor(
                                acc[:], acc[:], h[:, 0:1], mybir.AluOpType.add)
                            continue
                        for g in range(4):
                            ptr = p4ps.tile([128, 512], f32, tag="ptr")
                            for q in range(4):
                                nc.tensor.transpose(
                                    ptr[:, q * 128:(q + 1) * 128],
                                    h[:, (g * 4 + q) * 128:(g * 4 + q + 1) * 128],
                                    ident[:])
                            hT = p4t.tile([128, 512], bf16, tag="hT")
                            if g % 2 == 0:
                                nc.vector.tensor_copy(hT[:], ptr[:])
                            else:
                                nc.scalar.activation(
                                    hT[:], ptr[:], mybir.ActivationFunctionType.Copy)
                            for q in range(4):
                                t_idx = cc * 16 + g * 4 + q
                                first = (cc == 0 and g == 0 and q == 0)
                                last = (cc == NCC - 1 and g == 3 and q == 3)
                                nc.tensor.matmul(
                                    po[:],
                                    hT[:, q * 128:(q + 1) * 128],
                                    eft1[:, t_idx * W:(t_idx + 1) * W],
                                    start=first, stop=last, skip_group_check=True)
                    if dma_only:
                        continue
                    dn = p4o.tile([128, 1], f32, tag="dn")
                    nc.vector.tensor_scalar_max(dn[:], po[:, OUT_CH:W], 1e-20)
                    rec = p4o.tile([128, 1], f32, tag="rec")
                    nc.vector.reciprocal(rec[:], dn[:])
                    ot = p4o.tile([128, OUT_CH], f32, tag="ot")
                    nc.vector.tensor_scalar_mul(ot[:], po[:, 0:OUT_CH], rec[:])
                    nc.sync.dma_start(out_ext[nt * 128:(nt + 1) * 128, :], ot[:])

        if dma_only or 4 not in phases:
            # make sure something reaches the output so nothing is DCE'd
            ot = constp.tile([128, OUT_CH], f32)
            nc.vector.memset(ot[:], 0.0)
            nc.vector.tensor_tensor(
                ot[:, 0:1], ot[:, 0:1], acc[:], mybir.AluOpType.add)
            if 3 in phases and not dma_only:
                nc.vector.tensor_tensor(
                    ot[:, 0:1], ot[:, 0:1], eft1[:, 0:1], mybir.AluOpType.add)
            if 1 in phases and not dma_only and 3 not in phases:
                nc.vector.tensor_tensor(
                    ot[:, 0:1], ot[:, 0:1], xp1[:, 0:1], mybir.AluOpType.add)
            nc.sync.dma_start(out_ext[0:128, :], ot[:])


def build_graph(reps=1, phases=(0, 1, 2, 3, 4), dma_only=False):
    nc = bacc.Bacc("TRN2", target_bir_lowering=False, debug=False,
                   num_devices=N_CORES)
    x_ext = nc.dram_tensor("x", [NL, IN_CH], f32, kind="ExternalInput")
    H_ext = nc.dram_tensor("H", [NL, EP], f32, kind="ExternalInput")
    th_ext = nc.dram_tensor("theta", [IN_CH, OUT_CH], f32, kind="ExternalInput")
    id_ext = nc.dram_tensor("ident", [128, 128], f32, kind="ExternalInput")
    out_ext = nc.dram_tensor("out", [NL, OUT_CH], f32, kind="ExternalOutput")
    with tile.TileContext(nc) as tc:
        for r in range(reps):
            build_body(nc, tc, x_ext, H_ext, th_ext, id_ext, out_ext,
                       sfx=str(r), phases=phases, dma_only=dma_only)
    nc.compile()
    return nc


def make_in_maps(x, H, theta):
    x_pad = np.zeros((NL * N_CORES, IN_CH), np.float32)
    x_pad[:N] = x
    H_pad = np.zeros((NL * N_CORES, EP), np.float32)
    H_pad[:N, :E] = H
    ident = np.eye(128, dtype=np.float32)
    theta = np.asarray(theta, np.float32)
    in_maps = []
    for c in range(N_CORES):
        in_maps.append({
            "x": x_pad[c * NL:(c + 1) * NL],
            "H": H_pad[c * NL:(c + 1) * NL],
            "theta": theta,
            "ident": ident,
        })
    return in_maps


def kernel(x, H, theta):
    x = np.asarray(x, np.float32)
    H = np.asarray(H, np.float32)
    nc = build_graph(reps=1)
    in_maps = make_in_maps(x, H, theta)
    res = run_bass_kernel_spmd(nc, in_maps, core_ids=list(range(N_CORES)))
    out = np.concatenate(
        [res.results[c]["out"] for c in range(N_CORES)], axis=0)
    return np.ascontiguousarray(out[:N])


if __name__ == "__main__":
    rng = np.random.default_rng(0)
    x = rng.standard_normal((N, IN_CH), dtype=np.float32)
    H = rng.random((N, E), dtype=np.float32)
    theta = (rng.standard_normal((IN_CH, OUT_CH), dtype=np.float32)
             / np.sqrt(IN_CH))
    out = kernel(x, H, theta)
    xp = x @ theta
    de = H.sum(0)
    eft = (H.T @ xp) / de[:, None]
    dn = H.sum(1)
    ref = (H @ eft) / dn[:, None]
    err = np.abs(out - ref).max() / np.abs(ref).max()
    print("rel err:", err)



# revision 4
# speedup vs baseline: 2.0830x; 2.0830x over previous
"""DAHH hypergraph conv (gnn_message_passing) on 8 Trainium2 NeuronCores.

reference:
    xp      = x @ theta                      [N, 64]
    de      = colsum(H)                      [E]
    edge_ft = (H.T @ xp) / de[:, None]       [E, 64]
    dn      = rowsum(H)                      [N]
    node_ft = (H @ edge_ft) / dn[:, None]    [N, 64]

Sharding: H and x row-sharded (node dim) across 8 cores; theta replicated.

Single-read pipelined design (v2): H is read from HBM ONCE per core
(100 MB f32). The edge dim is split into NCH chunks; per chunk c:
  - 20 node-tile DMAs land H[k, chunk] f32, cast to bf16 (DVE/ACT split)
  - phase-1 matmuls: psum[e,65] += H_tile.T @ [xp|1]  (H bf16 as PE weights,
    FWL; edges directly on psum partitions -- no transposes needed)
  - the same bf16 tile is block-transposed by the DMA XBAR (one
    dma_start(transpose=True) per node tile, 3D out AP) into a persistent
    SBUF hT chunk buffer [e,n] -- no PE/DVE cost
  - chunk AllReduce (f32, 266 KB) over the 8 node shards via DRAM bounce
  - normalize -> eft1 chunk [e, 64|1] bf16
  - phase-4 matmuls: po[n, 65] += hT_block.T @ eft1  accumulated in packed
    psum slices across all chunks
Software-pipelined with depth D=2: phase-4 of chunk c is emitted after
phase-1 of chunk c+2, so the per-chunk AllReduce latency hides under two
chunks of phase-1 work. hT chunk buffers rotate 3-deep.

N padded 20000->20480 (2560/core), E padded 10000->10240 with zeros;
padding is numerically inert (zero rows/cols, degree clamps via
max(.,1e-20)).
"""
import sys
sys.path.insert(0, "/opt/trn_rl_repo")
import numpy as np

import concourse.bass as bass
import concourse.bacc as bacc
import concourse.tile as tile
import concourse.mybir as mybir
from concourse.bass_utils import run_bass_kernel_spmd

N, E, IN_CH, OUT_CH = 20000, 10000, 128, 64
N_CORES = 8
NL = 2560            # padded nodes per core
EP = 10240           # padded edges
NT = NL // 128       # 20 node tiles per core
ET = EP // 128       # 80 edge tiles
W = 65               # 64 features + degree/ones column

CCOLS = 1024         # edge cols per chunk
CET = CCOLS // 128   # 8 edge tiles per chunk
NCH = EP // CCOLS    # 10 chunks
D = 2                # software pipeline depth (phase4 lag in chunks)
HTBUFS = 3           # hT chunk buffers in flight

f32 = mybir.dt.float32
bf16 = mybir.dt.bfloat16

SKIP_COLLECTIVE = False   # dev-only: replace AllReduce with local copy
USE_XBAR = True           # False: PE transposes instead of DMA XBAR


def build_body(nc, tc, x_ext, H_ext, th_ext, id_ext, out_ext, sfx=""):
    PACK = 7  # po accumulators packed per psum bank
    with (
        tc.tile_pool(name=f"const{sfx}", bufs=1) as constp,
        tc.tile_pool(name=f"persist{sfx}", bufs=1) as persist,
        tc.tile_pool(name=f"dram{sfx}", bufs=1, space="DRAM") as dram,
    ):
        ident = constp.tile([128, 128], f32)
        nc.sync.dma_start(ident[:], id_ext[:])
        identb = constp.tile([128, 128], bf16)
        nc.vector.tensor_copy(identb[:], ident[:])
        th_f = constp.tile([128, OUT_CH], f32)
        nc.sync.dma_start(th_f[:], th_ext[:])
        th_b = constp.tile([128, OUT_CH], bf16)
        nc.vector.tensor_copy(th_b[:], th_f[:])

        xp1 = persist.tile([128, NT * W], bf16)

        # ---- phase 0: xp1 = [x @ theta | 1] per node tile ----
        with (
            tc.tile_pool(name=f"p0{sfx}", bufs=3) as p0,
            tc.tile_pool(name=f"p0ps{sfx}", bufs=2, space="PSUM") as p0ps,
        ):
            for k in range(NT):
                xt = p0.tile([128, 128], f32, tag="x")
                nc.sync.dma_start(xt[:], x_ext[k * 128:(k + 1) * 128, :])
                pt = p0ps.tile([128, 128], f32, tag="pt")
                nc.tensor.transpose(pt[:], xt[:], ident[:])
                xT = p0.tile([128, 128], bf16, tag="xT")
                nc.vector.tensor_copy(xT[:], pt[:])
                pxp = p0ps.tile([128, OUT_CH], f32, tag="pxp")
                nc.tensor.matmul(pxp[:], xT[:], th_b[:], start=True, stop=True)
                nc.vector.tensor_copy(xp1[:, k * W:k * W + OUT_CH], pxp[:])
                nc.vector.memset(xp1[:, k * W + OUT_CH:(k + 1) * W], 1.0)

        # ---- main pipelined loop over edge chunks ----
        bins, bouts = [], []
        for c in range(NCH):
            bins.append(dram.tile([128, CET * W], f32,
                                  name=f"bin{c}{sfx}", tag=f"bin{c}"))
            bouts.append(dram.tile([128, CET * W], f32, addr_space="Shared",
                                   name=f"bout{c}{sfx}", tag=f"bout{c}"))

        with (
            tc.tile_pool(name=f"hT{sfx}", bufs=HTBUFS) as hTp,
            tc.tile_pool(name=f"hf{sfx}", bufs=4) as hfp,
            tc.tile_pool(name=f"hb{sfx}", bufs=4) as hbp,
            tc.tile_pool(name=f"ar{sfx}", bufs=2) as arp,
            tc.tile_pool(name=f"ef{sfx}", bufs=2) as efp,
            tc.tile_pool(name=f"p1ps{sfx}", bufs=2, space="PSUM") as p1ps,
            tc.tile_pool(name=f"pops{sfx}", bufs=1, space="PSUM") as pops,
            tc.tile_pool(name=f"ptps{sfx}", bufs=2, space="PSUM") as ptps,
            tc.tile_pool(name=f"outp{sfx}", bufs=3) as outp,
        ):
            po_packs = [
                pops.tile([128, min(PACK, NT - i * PACK) * W], f32,
                          name=f"po{i}{sfx}", tag=f"po{i}")
                for i in range((NT + PACK - 1) // PACK)
            ]

            def po_slice(nt):
                return po_packs[nt // PACK][:, (nt % PACK) * W:
                                            (nt % PACK) * W + W]

            hT_bufs = {}

            def emit_phase1(c):
                hT_c = hTp.tile([128, CET * NT * 128], bf16, tag="hTc",
                                name=f"hTc{c}{sfx}")
                hT_bufs[c] = hT_c
                p1a = p1ps.tile([128, 4 * W], f32, tag="p1a",
                                name=f"p1a{c}{sfx}")
                p1b = p1ps.tile([128, 4 * W], f32, tag="p1b",
                                name=f"p1b{c}{sfx}")
                for k in range(NT):
                    hf = hfp.tile([128, CCOLS], f32, tag="hf",
                                  name=f"hf{c}_{k}{sfx}")
                    nc.sync.dma_start(
                        hf[:], H_ext[k * 128:(k + 1) * 128,
                                     c * CCOLS:(c + 1) * CCOLS])
                    hb = hbp.tile([128, CCOLS], bf16, tag="hb",
                                  name=f"hb{c}_{k}{sfx}")
                    if k % 2 == 0:
                        nc.vector.tensor_copy(hb[:], hf[:])
                    else:
                        nc.scalar.activation(
                            hb[:], hf[:], mybir.ActivationFunctionType.Copy)
                    if USE_XBAR:
                        dst = hT_c[:, k * CET * 128:(k + 1) * CET * 128]
                        nc.scalar.dma_start(
                            dst.rearrange("p (j n) -> p j n", n=128),
                            hb[:, :], transpose=True)
                    else:
                        for et in range(CET):
                            ptr = ptps.tile([128, 128], f32, tag="ptr",
                                            name=f"ptr{c}_{k}_{et}{sfx}")
                            nc.tensor.transpose(
                                ptr[:], hb[:, et * 128:(et + 1) * 128],
                                identb[:])
                            eng = nc.vector if et % 2 == 0 else nc.scalar
                            if et % 2 == 0:
                                nc.vector.tensor_copy(
                                    hT_c[:, (k * CET + et) * 128:
                                         (k * CET + et + 1) * 128], ptr[:])
                            else:
                                nc.scalar.activation(
                                    hT_c[:, (k * CET + et) * 128:
                                         (k * CET + et + 1) * 128], ptr[:],
                                    mybir.ActivationFunctionType.Copy)
                    for et in range(CET):
                        ps = p1a if et < 4 else p1b
                        # start=True zeroes the ENTIRE psum bank (measured),
                        # so only the first matmul touching each bank sets it;
                        # the other packed slices accumulate onto the zeroed
                        # bank.
                        nc.tensor.matmul(
                            ps[:, (et % 4) * W:(et % 4) * W + W],
                            hb[:, et * 128:(et + 1) * 128],
                            xp1[:, k * W:(k + 1) * W],
                            start=(k == 0 and et % 4 == 0),
                            stop=(k == NT - 1),
                            skip_group_check=True)
                ar_in = arp.tile([128, CET * W], f32, tag="arin",
                                 name=f"arin{c}{sfx}")
                nc.vector.tensor_copy(ar_in[:, 0:4 * W], p1a[:])
                nc.vector.tensor_copy(ar_in[:, 4 * W:8 * W], p1b[:])
                nc.sync.dma_start(bins[c][:], ar_in[:])
                if SKIP_COLLECTIVE:
                    nc.sync.dma_start(bouts[c][:], bins[c][:])
                else:
                    nc.gpsimd.collective_compute(
                        "AllReduce", mybir.AluOpType.add,
                        replica_groups=[list(range(N_CORES))],
                        ins=[bins[c].opt()], outs=[bouts[c].opt()])

            def emit_phase4(c):
                hT_c = hT_bufs.pop(c)
                eftf = efp.tile([128, CET * W], f32, tag="eftf",
                                name=f"eftf{c}{sfx}")
                nc.sync.dma_start(eftf[:], bouts[c][:])
                eft1 = efp.tile([128, CET * W], bf16, tag="eft1",
                                name=f"eft1{c}{sfx}")
                de = outp.tile([128, CET], f32, tag="de", name=f"de{c}{sfx}")
                nc.vector.tensor_scalar_max(
                    de[:], eftf[:, OUT_CH::W], 1e-20)
                rec = outp.tile([128, CET], f32, tag="rec",
                                name=f"rec{c}{sfx}")
                nc.vector.reciprocal(rec[:], de[:])
                for et in range(CET):
                    nc.vector.tensor_scalar_mul(
                        eft1[:, et * W:et * W + OUT_CH],
                        eftf[:, et * W:et * W + OUT_CH],
                        rec[:, et:et + 1])
                nc.vector.memset(eft1[:, OUT_CH::W], 1.0)
                for nt in range(NT):
                    for et in range(CET):
                        # bank-first matmul only (see phase-1 note): the three
                        # po packs are zeroed by nt 0/7/14's first matmul at
                        # chunk 0; everything else accumulates.
                        nc.tensor.matmul(
                            po_slice(nt),
                            hT_c[:, (nt * CET + et) * 128:
                                 (nt * CET + et + 1) * 128],
                            eft1[:, et * W:(et + 1) * W],
                            start=(c == 0 and et == 0 and nt % PACK == 0),
                            stop=(c == NCH - 1 and et == CET - 1),
                            skip_group_check=True)

            for s in range(NCH + D):
                if s < NCH:
                    emit_phase1(s)
                if s >= D:
                    emit_phase4(s - D)

            # ---- output: out = po[:, 0:64] / max(po[:, 64], eps) ----
            for nt in range(NT):
                po = po_slice(nt)
                dn = outp.tile([128, 1], f32, tag="dn", name=f"dn{nt}{sfx}")
                nc.vector.tensor_scalar_max(dn[:], po[:, OUT_CH:W], 1e-20)
                rcn = outp.tile([128, 1], f32, tag="rcn", name=f"rcn{nt}{sfx}")
                nc.vector.reciprocal(rcn[:], dn[:])
                ot = outp.tile([128, OUT_CH], f32, tag="ot",
                               name=f"ot{nt}{sfx}")
                nc.vector.tensor_scalar_mul(ot[:], po[:, 0:OUT_CH], rcn[:])
                nc.sync.dma_start(out_ext[nt * 128:(nt + 1) * 128, :], ot[:])


def build_graph(reps=1):
    nc = bacc.Bacc("TRN2", target_bir_lowering=False, debug=False,
                   num_devices=N_CORES)
    x_ext = nc.dram_tensor("x", [NL, IN_CH], f32, kind="ExternalInput")
    H_ext = nc.dram_tensor("H", [NL, EP], f32, kind="ExternalInput")
    th_ext = nc.dram_tensor("theta", [IN_CH, OUT_CH], f32, kind="ExternalInput")
    id_ext = nc.dram_tensor("ident", [128, 128], f32, kind="ExternalInput")
    out_ext = nc.dram_tensor("out", [NL, OUT_CH], f32, kind="ExternalOutput")
    with tile.TileContext(nc) as tc:
        for r in range(reps):
            build_body(nc, tc, x_ext, H_ext, th_ext, id_ext, out_ext,
                       sfx=str(r))
    nc.compile()
    return nc


def make_in_maps(x, H, theta):
    x_pad = np.zeros((NL * N_CORES, IN_CH), np.float32)
    x_pad[:N] = x
    H_pad = np.zeros((NL * N_CORES, EP), np.float32)
    H_pad[:N, :E] = H
    ident = np.eye(128, dtype=np.float32)
    theta = np.asarray(theta, np.float32)
    in_maps = []
    for c in range(N_CORES):
        in_maps.append({
            "x": x_pad[c * NL:(c + 1) * NL],
            "H": H_pad[c * NL:(c + 1) * NL],
            "theta": theta,
            "ident": ident,
        })
    return in_maps


def kernel(x, H, theta):
    x = np.asarray(x, np.float32)
    H = np.asarray(H, np.float32)
    nc = build_graph(reps=1)
    in_maps = make_in_maps(x, H, theta)
    res = run_bass_kernel_spmd(nc, in_maps, core_ids=list(range(N_CORES)))
    out = np.concatenate(
        [res.results[c]["out"] for c in range(N_CORES)], axis=0)
    return np.ascontiguousarray(out[:N])


if __name__ == "__main__":
    rng = np.random.default_rng(0)
    x = rng.standard_normal((N, IN_CH), dtype=np.float32)
    H = rng.random((N, E), dtype=np.float32)
    theta = (rng.standard_normal((IN_CH, OUT_CH), dtype=np.float32)
             / np.sqrt(IN_CH))
    out = kernel(x, H, theta)
    xp = x @ theta
    de = H.sum(0)
    eft = (H.T @ xp) / de[:, None]
    dn = H.sum(1)
    ref = (H @ eft) / dn[:, None]
    err = np.abs(out - ref).max() / np.abs(ref).max()
    print("rel err:", err)
